# revision 1
# baseline (speedup 1.0000x reference)
"""Trainium2 Bass kernel for nn_AttentionLayer_79293686218841.

Self-contained: builds an 8-core SPMD Bass/Tile kernel, shards the full
inputs host-side (batch x seq-half data parallel), runs on NeuronCores via
run_bass_kernel_spmd, and gathers the full output.
"""

import sys

try:
    import concourse.bass  # noqa: F401
except ImportError:
    sys.path.insert(0, "/opt/trn_rl_repo")



import numpy as np
import ml_dtypes

import concourse.bass as bass
import concourse.mybir as mybir
import concourse.tile as tile
from concourse import bacc

f32 = mybir.dt.float32
bf16 = mybir.dt.bfloat16
AF = mybir.ActivationFunctionType
OP = mybir.AluOpType

B, N, D = 4, 2048, 512
H, DH = 8, 64
NI = 1024      # queries per core
P = 128
NJT = N // P   # 16 j-chunks


def build_nc(reps: int = 1, interleave: bool = True):
    nc = bacc.Bacc("TRN2", target_bir_lowering=False, debug=False, num_devices=8)

    x_d = nc.dram_tensor("x", [N, D], bf16, kind="ExternalInput")
    xq_d = nc.dram_tensor("xq", [NI, D], bf16, kind="ExternalInput")
    mk_d = nc.dram_tensor("mk", [P, NJT], f32, kind="ExternalInput")
    mqi_d = nc.dram_tensor("mqi", [NI], f32, kind="ExternalInput")
    mqc_d = nc.dram_tensor("mqc", [NI], bf16, kind="ExternalInput")
    wq_d = nc.dram_tensor("Wq", [D, D], bf16, kind="ExternalInput")
    wk_d = nc.dram_tensor("Wk", [D, D], bf16, kind="ExternalInput")
    wv_d = nc.dram_tensor("Wv", [D, D], bf16, kind="ExternalInput")
    wo_d = nc.dram_tensor("Wo", [D, D], bf16, kind="ExternalInput")
    bo_d = nc.dram_tensor("bo", [D], f32, kind="ExternalInput")
    out_d = nc.dram_tensor("out", [NI, D], f32, kind="ExternalOutput")
    # dummy input whose shape depends on `reps`: defeats the weak NEFF-cache
    # key (which hashes only the wrapper HLO, not the embedded BIR)
    nc.dram_tensor("pad", [reps, 1], f32, kind="ExternalInput")

    with tile.TileContext(nc) as tc:
        with (
            tc.tile_pool(name="cw", bufs=1) as cw,
            tc.tile_pool(name="cst", bufs=1) as cst,
            tc.tile_pool(name="xp", bufs=1) as xp,
            tc.tile_pool(name="kqv", bufs=1) as kqv,
            tc.tile_pool(name="pT", bufs=6) as pTp,
            tc.tile_pool(name="rows", bufs=2) as rows,
            tc.tile_pool(name="nrm", bufs=2) as nrm,
            tc.tile_pool(name="osb", bufs=2) as osb,
            tc.tile_pool(name="psS", bufs=2, space="PSUM") as psS,
            tc.tile_pool(name="psOV", bufs=1, space="PSUM") as psOV,
            tc.tile_pool(name="psMM", bufs=2, space="PSUM") as psMM,
        ):
            # ---- constants / weights ----
            w_sb = {}
            for nm, d_ in (("wq", wq_d), ("wk", wk_d), ("wv", wv_d), ("wo", wo_d)):
                t = cw.tile([P, 4, D], bf16, name=nm)
                nc.sync.dma_start(t[:], d_.rearrange("(co p) d -> p co d", p=P))
                w_sb[nm] = t
            mk_sb = cst.tile([P, NJT], f32, name="mk_sb")
            nc.sync.dma_start(mk_sb[:], mk_d[:, :])
            mqi_r = cst.tile([1, NI], f32, name="mqi_r")
            nc.sync.dma_start(mqi_r[:], mqi_d[None, :])
            mqc_r = cst.tile([1, NI], bf16, name="mqc_r")
            nc.sync.dma_start(mqc_r[:], mqc_d[None, :])
            bo_f = cst.tile([1, D], f32, name="bo_f")
            nc.sync.dma_start(bo_f[:], bo_d[None, :])
            ones_r = cst.tile([1, P], bf16, name="ones_r")
            nc.vector.memset(ones_r[:], 1.0)
            bo_hi = cst.tile([1, D], bf16, name="bo_hi")
            nc.vector.tensor_copy(bo_hi[:], bo_f[:])
            bo_lo_f = cst.tile([1, D], f32, name="bo_lo_f")
            nc.vector.tensor_tensor(bo_lo_f[:], bo_f[:], bo_hi[:], OP.subtract)
            bo_lo = cst.tile([1, D], bf16, name="bo_lo")
            nc.vector.tensor_copy(bo_lo[:], bo_lo_f[:])

            for rep in range(reps):
                # ---- transposes ----
                xT = xp.tile([P, 4, N], bf16, name="xT")
                xqT = xp.tile([P, 4, NI], bf16, name="xqT")
                for cc in range(4):
                    nc.sync.dma_start_transpose(xT[:, cc, :], x_d[:, cc * P:(cc + 1) * P])
                    nc.sync.dma_start_transpose(xqT[:, cc, :], xq_d[:, cc * P:(cc + 1) * P])

                kT = kqv.tile([P, 4, N], bf16, name="kT")
                qT = kqv.tile([P, 4, NI], bf16, name="qT")
                v_sb = kqv.tile([P, NJT, H * 65], bf16, name="v_sb")
                v65 = v_sb[:].rearrange("p jo (h e) -> p jo h e", e=65)
                oT = kqv.tile([P, 4, NI], bf16, name="oT")

                # ---- emit-closures for projection work (interleavable) ----
                def kt_group(dc, nt):
                    ps = psMM.tile([P, 512], f32, name="mm")
                    for cc in range(4):
                        nc.tensor.matmul(
                            ps[:], w_sb["wk"][:, cc, dc * P:(dc + 1) * P],
                            xT[:, cc, nt * 512:(nt + 1) * 512],
                            start=(cc == 0), stop=(cc == 3))
                    nc.vector.tensor_copy(kT[:, dc, nt * 512:(nt + 1) * 512], ps[:])

                def qt_group(dc, nt):
                    ps = psMM.tile([P, 512], f32, name="mm")
                    for cc in range(4):
                        nc.tensor.matmul(
                            ps[:], w_sb["wq"][:, cc, dc * P:(dc + 1) * P],
                            xqT[:, cc, nt * 512:(nt + 1) * 512],
                            start=(cc == 0), stop=(cc == 3))
                    nc.vector.tensor_copy(qT[:, dc, nt * 512:(nt + 1) * 512], ps[:])

                def v_group(jt, half):
                    ps = psMM.tile([P, 512], f32, name="mm")
                    hsl = slice(half * 256, (half + 1) * 256)
                    for cc in range(4):
                        nc.tensor.matmul(
                            ps[:, 0:256], xT[:, cc, jt * P:(jt + 1) * P],
                            w_sb["wv"][:, cc, hsl],
                            start=(cc == 0), stop=(cc == 3))
                    # rows of masked keys -> 0 (mask folded into the drain copy)
                    nc.vector.tensor_scalar(
                        v65[:, jt, 4 * half:4 * half + 4, 0:64],
                        ps[:, 0:256].rearrange("p (h dd) -> p h dd", h=4),
                        mk_sb[:, jt:jt + 1], None, OP.mult)
                    # the "[V|1]" ones column is the mask itself
                    nc.vector.tensor_copy(
                        v65[:, jt, 4 * half:4 * half + 4, 64],
                        mk_sb[:, jt:jt + 1].to_broadcast((P, 4)))

                def mean_group():
                    # mean over keys of V = (sum_j x[j]) @ Wv (mask NOT applied:
                    # the reference's uniform fallback averages over all keys)
                    mxT_f = rows.tile([P, 4], f32, name="mxT_f")
                    nc.vector.reduce_sum(mxT_f[:], xT[:], axis=mybir.AxisListType.X)
                    mxT = rows.tile([P, 4], bf16, name="mxT")
                    nc.vector.tensor_copy(mxT[:], mxT_f[:])
                    mv_ps = psMM.tile([1, D], f32, name="mm")
                    for cc in range(4):
                        nc.tensor.matmul(mv_ps[:], mxT[:, cc:cc + 1],
                                         w_sb["wv"][:, cc, :],
                                         start=(cc == 0), stop=(cc == 3))
                    mean_bf = cst.tile([1, D], bf16, name="mean_bf")
                    nc.vector.tensor_copy(mean_bf[:], mv_ps[:])
                    return mean_bf

                bg = []  # background emission queue, popped inside attention
                # upfront: K/Q d-chunk 0 and V heads 0..3 (+ mean)
                for nt in range(4):
                    kt_group(0, nt)
                for nt in range(2):
                    qt_group(0, nt)
                for jt in range(NJT):
                    v_group(jt, 0)
                mean_bf = mean_group()
                for dc in range(1, 4):
                    for nt in range(4):
                        bg.append((kt_group, (dc, nt)))
                    for nt in range(2):
                        bg.append((qt_group, (dc, nt)))
                for jt in range(NJT):
                    bg.append((v_group, (jt, 1)))
                if not interleave:
                    while bg:
                        f, a = bg.pop(0)
                        f(*a)

                # ---- attention ----
                for hp in range(4):
                    h0, h1 = 2 * hp, 2 * hp + 1
                    for it2 in range(2):
                        isl = slice(it2 * 512, (it2 + 1) * 512)
                        ov0 = psOV.tile([P, 512], f32, name="ov0")
                        ov1 = psOV.tile([P, 512], f32, name="ov1")
                        pts = []
                        for jt in range(NJT):
                            s = psS.tile([P, 2, 512], f32, name="s")
                            nc.tensor.matmul(
                                s[:, 0, :], kT[0:64, hp, jt * P:(jt + 1) * P],
                                qT[0:64, hp, isl], start=True, stop=True,
                                tile_position=(0, 0))
                            nc.tensor.matmul(
                                s[:, 1, :], kT[64:128, hp, jt * P:(jt + 1) * P],
                                qT[64:128, hp, isl], start=True, stop=True,
                                tile_position=(64, 0))
                            p = pTp.tile([P, 2, 512], bf16, name="p")
                            nc.scalar.activation(p[:], s[:], AF.Exp, scale=0.125)
                            pts.append(p)
                            # consume previous chunk's P while this chunk's exp runs
                            if jt >= 1:
                                q = pts[jt - 1]
                                nc.tensor.matmul(
                                    ov0[0:65, :], v65[:, jt - 1, h0, :], q[:, 0, :],
                                    start=(jt - 1 == 0), stop=False)
                                nc.tensor.matmul(
                                    ov1[0:65, :], v65[:, jt - 1, h1, :], q[:, 1, :],
                                    start=(jt - 1 == 0), stop=False)
                            if bg and jt % 2 == 0:
                                f, a = bg.pop(0)
                                f(*a)
                        q = pts[-1]
                        nc.tensor.matmul(ov0[0:65, :], v65[:, NJT - 1, h0, :],
                                         q[:, 0, :], start=False, stop=True)
                        nc.tensor.matmul(ov1[0:65, :], v65[:, NJT - 1, h1, :],
                                         q[:, 1, :], start=False, stop=True)

                        for h, ov in ((h0, ov0), (h1, ov1)):
                            dn = rows.tile([1, 512], f32, name="dn")
                            nc.vector.tensor_tensor(dn[:], ov[64:65, :],
                                                    mqi_r[0:1, isl], OP.add)
                            rc = rows.tile([1, 512], f32, name="rc")
                            nc.vector.reciprocal(rc[:], dn[:])
                            rc_hi = rows.tile([1, 512], bf16, name="rc_hi")
                            nc.vector.tensor_copy(rc_hi[:], rc[:])
                            rc_lo_f = rows.tile([1, 512], f32, name="rc_lo_f")
                            nc.vector.tensor_tensor(rc_lo_f[:], rc[:], rc_hi[:], OP.subtract)
                            rc_lo = rows.tile([1, 512], bf16, name="rc_lo")
                            nc.vector.tensor_copy(rc_lo[:], rc_lo_f[:])
                            b1 = psMM.tile([64, 512], f32, name="mm")
                            nc.tensor.matmul(b1[:], ones_r[0:1, 0:64], rc_hi[:],
                                             start=True, stop=False)
                            nc.tensor.matmul(b1[:], ones_r[0:1, 0:64], rc_lo[:],
                                             start=False, stop=True)
                            b1s = nrm.tile([64, 512], f32, name="b1s")
                            nc.vector.tensor_copy(b1s[:], b1[:])
                            b2 = psMM.tile([64, 512], f32, name="mm")
                            nc.tensor.matmul(b2[:], mean_bf[0:1, h * DH:(h + 1) * DH],
                                             mqc_r[0:1, isl], start=True, stop=True)
                            t1 = nrm.tile([64, 512], f32, name="t1")
                            nc.vector.tensor_mul(t1[:], ov[0:64, :], b1s[:])
                            dst = oT[64 * (h % 2):64 * (h % 2) + 64, h // 2, isl]
                            nc.vector.tensor_tensor(dst, t1[:], b2[:], OP.add)

                # ---- output projection ----
                for ic in range(8):
                    fp = psMM.tile([P, 512], f32, name="mm")
                    for cc in range(4):
                        nc.tensor.matmul(fp[:], oT[:, cc, ic * P:(ic + 1) * P],
                                         w_sb["wo"][:, cc, :],
                                         start=(cc == 0), stop=False)
                    nc.tensor.matmul(fp[:], ones_r[0:1, :], bo_hi[:],
                                     start=False, stop=False)
                    nc.tensor.matmul(fp[:], ones_r[0:1, :], bo_lo[:],
                                     start=False, stop=True)
                    o = osb.tile([P, 512], f32, name="o")
                    nc.vector.tensor_copy(o[:], fp[:])
                    nc.sync.dma_start(out_d[ic * P:(ic + 1) * P, :], o[:])

    nc.compile()
    return nc


def make_in_maps(x, mask_k, mask_q, Wq, Wk, Wv, Wo, bo, reps=1):
    """Shard full inputs into 8 per-core input maps (host-side marshaling)."""
    bf = ml_dtypes.bfloat16
    x_bf = x.astype(bf)
    w = {"Wq": Wq.astype(bf), "Wk": Wk.astype(bf), "Wv": Wv.astype(bf),
         "Wo": Wo.astype(bf), "bo": bo.astype(np.float32)}
    in_maps = []
    for c in range(8):
        b, hf = c // 2, c % 2
        qsl = slice(hf * NI, (hf + 1) * NI)
        mk = mask_k[b].astype(np.float32)
        mq = mask_q[b, qsl].astype(np.float32)
        in_maps.append({
            "pad": np.zeros((reps, 1), np.float32),
            "x": np.ascontiguousarray(x_bf[b]),
            "xq": np.ascontiguousarray(x_bf[b, qsl]),
            "mk": np.ascontiguousarray(mk.reshape(NJT, P).T),
            "mqi": ((1.0 - mq) * 1e30).astype(np.float32),
            "mqc": ((1.0 - mq) / N).astype(bf),
            **w,
        })
    return in_maps


def assemble_out(results):
    out = np.empty((B, N, D), dtype=np.float32)
    for c in range(8):
        b, hf = c // 2, c % 2
        out[b, hf * NI:(hf + 1) * NI, :] = results[c]["out"]
    return out


_NC_CACHE = {}


def kernel(x, mask_k, mask_q, Wq, Wk, Wv, Wo, bo):
    from concourse.bass_utils import run_bass_kernel_spmd

    x = np.asarray(x, dtype=np.float32)
    mask_k = np.asarray(mask_k)
    mask_q = np.asarray(mask_q)
    Wq = np.asarray(Wq, dtype=np.float32)
    Wk = np.asarray(Wk, dtype=np.float32)
    Wv = np.asarray(Wv, dtype=np.float32)
    Wo = np.asarray(Wo, dtype=np.float32)
    bo = np.asarray(bo, dtype=np.float32)

    if "nc" not in _NC_CACHE:
        _NC_CACHE["nc"] = build_nc(reps=1)
    nc = _NC_CACHE["nc"]
    in_maps = make_in_maps(x, mask_k, mask_q, Wq, Wk, Wv, Wo, bo, reps=1)
    res = run_bass_kernel_spmd(nc, in_maps, core_ids=list(range(8)))
    return assemble_out(res.results)

